# revision 29
# baseline (speedup 1.0000x reference)
"""Multi-head causal attention (B=4, S=2048, D=1024, H=16) on 8 NeuronCores.

Sharding: core i handles batch b=i//2 and head-group g=i%2 (8 of 16 heads).
Tensor-parallel over heads: W_q/W_k/W_v column-sliced, W_o row-sliced; the
all-reduce after W_o is a host-side sum of the two partial outputs per batch.

v2: single fused schedule. The v1 kernel ran two serial phases: (1) all QKV
projections (PE-dense, ACT idle, ~103us) and (2) attention (ACT exp-paced at
~1147ns per [128,1024] tile while PE only had ~650ns of work per iteration,
~213us). Here the projection/W_o matmuls are fed through a work queue and
interleaved between attention iterations, so the PE's idle time under the
ACT pacing is spent on projections and both engines stay busy:

  - per query-window w: attention(w) iterations interleave matmuls from
    QKV(w+1) + W_o(w-1) drawn from the queue at a per-window fill rate.
  - scores pairs (K=64) pack via base_partition row-tiling as before.
  - softmax denominators still ride the AV matmul via a ones-column in V
    (psum row 64); but the reciprocal+broadcast is now: DVE reciprocal of
    the [1,512] denominator rows straight from PSUM, then GPSIMD
    partition_broadcast into [128,512] — eliminating v1's 64 plain-fp32
    K=1 broadcast matmuls (~28us of PE wall).
  - normalization multiplies read the raw context directly from PSUM
    (h1 via a partition-shift DMA as before), removing two DVE copies.
Predicted ~235us vs v1's 360us.
"""

from collections import deque

import numpy as np

import concourse.tile as tile
from concourse import bacc, bass_utils, mybir

F32 = mybir.dt.float32
F32R = mybir.dt.float32r
BF16 = mybir.dt.bfloat16
AF = mybir.ActivationFunctionType

B = 4
S = 2048
D = 1024
DH = 64
E = 512          # local e-width (8 heads x 64)
NW = 4           # 512-wide query windows
WQ = 512
NDC = 8          # 128-wide d-model chunks
NEC = 4          # 128-wide local-e chunks (head pairs)
N_CORES = 8
LAG = 4          # AV lags scores/exp by this many kc iterations
# proj-matmul fill per attention iteration, per window
RATES = {0: 10.0, 1: 3.2, 2: 2.8, 3: 1.6}

_cache = {}


def build_program():
    nc = bacc.Bacc(trn_type="TRN2", target_bir_lowering=False, debug=False)
    xt = nc.dram_tensor("xt", [D, S], BF16, kind="ExternalInput").ap()
    wq = nc.dram_tensor("wq", [D, E], BF16, kind="ExternalInput").ap()
    wk = nc.dram_tensor("wk", [D, E], BF16, kind="ExternalInput").ap()
    wv = nc.dram_tensor("wv", [D, E], BF16, kind="ExternalInput").ap()
    wo = nc.dram_tensor("wo", [E, D], BF16, kind="ExternalInput").ap()
    bq = nc.dram_tensor("bq", [E], F32, kind="ExternalInput").ap()
    bk = nc.dram_tensor("bk", [E], F32, kind="ExternalInput").ap()
    mk = nc.dram_tensor("mk", [128, 4 * WQ], BF16, kind="ExternalInput").ap()
    out = nc.dram_tensor("out", [S, D], F32, kind="ExternalOutput").ap()

    with tile.TileContext(nc) as tc:
        with (
            tc.tile_pool(name="const", bufs=1) as constp,
            tc.tile_pool(name="persist", bufs=1) as pers,
            tc.tile_pool(name="xtp", bufs=16) as xtp,
            tc.tile_pool(name="qtp", bufs=16) as qtp,
            tc.tile_pool(name="expp", bufs=6) as expp,
            tc.tile_pool(name="recp", bufs=2) as recp,
            tc.tile_pool(name="rbp", bufs=2) as rbp,
            tc.tile_pool(name="ctxp", bufs=6) as ctxp,
            tc.tile_pool(name="outp", bufs=2) as outp,
            tc.tile_pool(name="ppp", bufs=2, space="PSUM") as ppp,
            tc.tile_pool(name="spp", bufs=2, space="PSUM") as spp,
            tc.tile_pool(name="pcp", bufs=1, space="PSUM") as pcp,
        ):
            # ---- prefetch DMAs, ordered so Q(0,ec0)/K(0,ec0) unblock first:
            # xt(0) chunks, wq/wk ec0 columns, wv (for V groups, needed ~4
            # iters in), then the rest of wq/wk, biases, mask, wo.
            wq_s = constp.tile([128, NDC, E], BF16, name="wq_s")
            wk_s = constp.tile([128, NDC, E], BF16, name="wk_s")
            wv_s = constp.tile([128, NDC, E], BF16, name="wv_s")
            bq_s = constp.tile([128, NEC], F32, name="bq_s")
            bk_s = constp.tile([128, NEC], F32, name="bk_s")
            xts0 = []
            for dc in range(NDC):
                xtt = xtp.tile([128, WQ], BF16, name=f"xt_0_{dc}", tag="xt")
                nc.sync.dma_start(xtt[:], xt[dc * 128 : (dc + 1) * 128, 0:WQ])
                xts0.append(xtt)
                nc.sync.dma_start(
                    wq_s[:, dc, 0:128], wq[dc * 128 : (dc + 1) * 128, 0:128]
                )
                nc.sync.dma_start(
                    wk_s[:, dc, 0:128], wk[dc * 128 : (dc + 1) * 128, 0:128]
                )
            nc.sync.dma_start(bq_s[:], bq.rearrange("(c p) -> p c", p=128))
            nc.sync.dma_start(bk_s[:], bk.rearrange("(c p) -> p c", p=128))
            for dc in range(NDC):
                nc.sync.dma_start(wv_s[:, dc, :], wv[dc * 128 : (dc + 1) * 128, :])
            for dc in range(NDC):
                nc.sync.dma_start(
                    wq_s[:, dc, 128:E], wq[dc * 128 : (dc + 1) * 128, 128:E]
                )
                nc.sync.dma_start(
                    wk_s[:, dc, 128:E], wk[dc * 128 : (dc + 1) * 128, 128:E]
                )
            mask_s = constp.tile([128, 4, WQ], BF16, name="mask_s")
            nc.sync.dma_start(mask_s[:], mk.rearrange("p (r j) -> p r j", r=4))
            # wo_s is DMA'd later (after xt(1)) — it is only needed for the
            # W_o(0) fills during ATT(1), and front-loading it delays the
            # xt(1) chunks that Q(1) fill matmuls stall the PE queue on.
            wo_s = constp.tile([128, NEC, D], BF16, name="wo_s")

            # K^T [e, s] and V(+ones) [s, 8*(64+1)] persistent, bf16
            KT = pers.tile([128, NEC, S], BF16, name="KT")
            VP = pers.tile([128, 16, 520], BF16, name="VP")
            for sc16 in range(16):
                ones_ap = VP[:, sc16, :].rearrange("p (h c) -> p h c", h=8)[:, :, 64:65]
                nc.vector.memset(ones_ap, 1.0)
            ones_b = constp.tile([1, 64], BF16, name="ones_b")
            nc.vector.memset(ones_b[:], 1.0)

            qts_all = [[None] * NEC for _ in range(NW)]

            # ==== projection work queue: generators, one matmul per next() ====
            proj_q = deque()  # (tag, generator)

            def gen_q_group(w, ec, xts):
                p = None
                for dc in range(NDC):
                    if dc:
                        yield
                    if p is None:
                        p = ppp.tile([128, WQ], F32, name=f"pq_{w}_{ec}", tag="pp")
                    nc.tensor.matmul(
                        p[:],
                        wq_s[:, dc, ec * 128 : (ec + 1) * 128],
                        xts[dc][:],
                        start=(dc == 0),
                        stop=(dc == NDC - 1),
                    )
                qt = qtp.tile([128, WQ], BF16, name=f"qt_{w}_{ec}", tag="qt")
                nc.vector.tensor_scalar_add(qt[:], p[:], bq_s[:, ec : ec + 1])
                qts_all[w][ec] = qt

            def gen_k_group(w, ec, xts):
                p = None
                for dc in range(NDC):
                    if dc:
                        yield
                    if p is None:
                        p = ppp.tile([128, WQ], F32, name=f"pk_{w}_{ec}", tag="pp")
                    nc.tensor.matmul(
                        p[:],
                        wk_s[:, dc, ec * 128 : (ec + 1) * 128],
                        xts[dc][:],
                        start=(dc == 0),
                        stop=(dc == NDC - 1),
                    )
                nc.vector.tensor_scalar_add(
                    KT[:, ec, w * WQ : (w + 1) * WQ], p[:], bk_s[:, ec : ec + 1]
                )

            def gen_v_group(w, sc, xts):
                p = None
                for dc in range(NDC):
                    if dc:
                        yield
                    if p is None:
                        p = ppp.tile([128, WQ], F32, name=f"pv_{w}_{sc}", tag="pp")
                    nc.tensor.matmul(
                        p[:],
                        xts[dc][:, sc * 128 : (sc + 1) * 128],
                        wv_s[:, dc, :],
                        start=(dc == 0),
                        stop=(dc == NDC - 1),
                    )
                sc16 = w * 4 + sc
                vdst = VP[:, sc16, :].rearrange("p (h c) -> p h c", h=8)[:, :, 0:64]
                nc.vector.tensor_copy(vdst, p[:].rearrange("p (h c) -> p h c", h=8))

            def gen_wo_group(w, sc, n2, ctx_w):
                p = None
                for ec in range(NEC):
                    if ec:
                        yield
                    if p is None:
                        p = ppp.tile([128, WQ], F32, name=f"po_{w}_{sc}_{n2}", tag="pp")
                    nc.tensor.matmul(
                        p[:],
                        ctx_w[ec][:, sc * 128 : (sc + 1) * 128],
                        wo_s[:, ec, n2 * WQ : (n2 + 1) * WQ],
                        start=(ec == 0),
                        stop=(ec == NEC - 1),
                    )
                ot = outp.tile([128, WQ], F32, name=f"ot_{w}_{sc}_{n2}", tag="ot")
                nc.vector.tensor_copy(ot[:], p[:])
                r0 = w * WQ + sc * 128
                nc.sync.dma_start(out[r0 : r0 + 128, n2 * WQ : (n2 + 1) * WQ], ot[:])

            def enqueue_qkv(w, xts):
                # hp i's first score needs Q(i)+K(i); V(sc) needed ~LAG in.
                if w == 0:
                    order = [("q", 0), ("k", 0)]
                    order += [("v", 0), ("q", 1), ("k", 1), ("v", 1), ("v", 2)]
                    order += [("v", 3), ("q", 2), ("k", 2), ("q", 3), ("k", 3)]
                else:
                    order = [("q", i) for i in range(NEC)]
                    order += [("k", i) for i in range(NEC)]
                    order += [("v", i) for i in range(NEC)]
                gens = {"q": gen_q_group, "k": gen_k_group, "v": gen_v_group}
                for kind, i in order:
                    proj_q.append((("qkv", w), gens[kind](w, i, xts)))

            def fill(n):
                while n > 0 and proj_q:
                    try:
                        next(proj_q[0][1])
                    except StopIteration:
                        proj_q.popleft()
                        continue
                    n -= 1

            def drain_tag(t):
                while proj_q and proj_q[0][0] == t:
                    try:
                        next(proj_q[0][1])
                    except StopIteration:
                        proj_q.popleft()

            enqueue_qkv(0, xts0)
            # prologue: emit Q(0,0)/K(0,0) so attention can start; the rest
            # of QKV(0) flows in as fill during ATT(0).
            for _ in range(2 * NDC):
                fill(1)

            acc = [0.0]
            clock = [15000.0]  # sim-time floor (ns) for fill scheduling

            def fill_iter(rate):
                acc[0] += rate
                n = int(acc[0])
                acc[0] -= n
                fill(n)

            # ==== fused attention + interleaved projections ====
            for w in range(NW):
                if w + 1 < NW:
                    xts = []
                    for dc in range(NDC):
                        xtt = xtp.tile(
                            [128, WQ], BF16, name=f"xt_{w + 1}_{dc}", tag="xt"
                        )
                        nc.sync.dma_start(
                            xtt[:],
                            xt[dc * 128 : (dc + 1) * 128, (w + 1) * WQ : (w + 2) * WQ],
                        )
                        xts.append(xtt)
                    enqueue_qkv(w + 1, xts)
                    if w == 0:
                        for ec in range(NEC):
                            nc.sync.dma_start(
                                wo_s[:, ec, :], wo[ec * 128 : (ec + 1) * 128, :]
                            )
                # correctness backstop: this window's projections must be in
                # the PE stream before attention reads them.
                drain_tag(("qkv", w))
                rate = RATES[w]
                nkc = 4 * w + 4
                ctx_w = []
                for hp in range(NEC):
                    c0 = pcp.tile([128, WQ], F32, name=f"c0_{w}_{hp}", tag="c0")
                    c1 = pcp.tile([128, WQ], F32, name=f"c1_{w}_{hp}", tag="c1")

                    def emit_av(kc, ex, nq, qo, c0=c0, c1=c1, hp=hp, nkc=nkc):
                        nc.tensor.matmul(
                            c0[0:65, qo:WQ],
                            VP[:, kc, (2 * hp) * 65 : (2 * hp) * 65 + 65],
                            ex[:, 0, 0:nq],
                            start=(kc == 0),
                            stop=(kc == nkc - 1),
                        )
                        nc.tensor.matmul(
                            c1[0:65, qo:WQ],
                            VP[:, kc, (2 * hp + 1) * 65 : (2 * hp + 1) * 65 + 65],
                            ex[:, 1, 0:nq],
                            start=(kc == 0),
                            stop=(kc == nkc - 1),
                        )

                    avq = []
                    for kc in range(nkc):
                        # trapezoid compaction: for diagonal chunks only the
                        # queries q >= r*128 can attend chunk kc — skip the
                        # causally-dead columns in scores/exp/AV entirely.
                        r = kc - 4 * w
                        qo = max(r, 0) * 128
                        nq = WQ - qo
                        sp = spp.tile(
                            [128, 2, WQ], F32, name=f"sp_{w}_{hp}_{kc}", tag="sp"
                        )
                        nc.tensor.matmul(
                            sp[:, 0, 0:nq],
                            KT[0:64, hp, kc * 128 : (kc + 1) * 128],
                            qts_all[w][hp][0:64, qo:WQ],
                            start=True,
                            stop=True,
                        )
                        nc.tensor.matmul(
                            sp[:, 1, 0:nq],
                            KT[64:128, hp, kc * 128 : (kc + 1) * 128],
                            qts_all[w][hp][64:128, qo:WQ],
                            start=True,
                            stop=True,
                        )
                        ex = expp.tile(
                            [128, 2, WQ], BF16, name=f"ex_{w}_{hp}_{kc}", tag="ex"
                        )
                        nc.scalar.activation(ex[:, :, 0:nq], sp[:, :, 0:nq], AF.Exp)
                        if r >= 0:
                            # causal mask on the 128-wide diagonal sub-block
                            # (compacted cols 0:128): keep j >= i
                            nc.vector.tensor_mul(
                                ex[:, 0, 0:128], ex[:, 0, 0:128], mask_s[:, 0, 0:128]
                            )
                            nc.vector.tensor_mul(
                                ex[:, 1, 0:128], ex[:, 1, 0:128], mask_s[:, 0, 0:128]
                            )
                        avq.append((kc, ex, nq, qo))
                        if len(avq) > LAG:
                            emit_av(*avq.pop(0))
                        # fills carry a sim-time floor so the scheduler
                        # interleaves them with attention instead of running
                        # every dep-free projection ahead of it
                        clock[0] += max((2 * nq + 352) / 1.2, 648 + rate * 216)
                        with tc.tile_wait_until(clock[0] / 1e6):
                            fill_iter(rate)
                    for item in avq:
                        emit_av(*item)
                    clock[0] += min(LAG, nkc) * 216 + 500
                    # ---- finalize: denominator rows -> bf16, raw ctx ->
                    # SBUF, then one col-packed PE broadcast slot (two
                    # concurrent M=64 tiles) REUSING c0's psum bank,
                    # reciprocal, normalize ----
                    dnb = recp.tile([1, 2, WQ], BF16, name=f"dnb_{w}_{hp}", tag="dnb")
                    nc.vector.tensor_copy(dnb[0:1, 0, :], c0[64:65, :])
                    nc.vector.tensor_copy(dnb[0:1, 1, :], c1[64:65, :])
                    cr0 = recp.tile([64, WQ], F32, name=f"cr0_{w}_{hp}", tag="cr0")
                    nc.vector.tensor_copy(cr0[:], c0[0:64, :])
                    cr1 = recp.tile([64, WQ], BF16, name=f"cr1_{w}_{hp}", tag="cr1")
                    nc.vector.tensor_copy(cr1[:], c1[0:64, :])
                    nc.tensor.matmul(
                        c0[0:64, :], ones_b[0:1, 0:64], dnb[0:1, 0, :],
                        start=True, stop=True, tile_position=(0, 0),
                    )
                    nc.tensor.matmul(
                        c0[64:128, :], ones_b[0:1, 0:64], dnb[0:1, 1, :],
                        start=True, stop=True, tile_position=(0, 64),
                    )
                    rb = rbp.tile([128, WQ], F32, name=f"rb_{w}_{hp}", tag="rb")
                    nc.vector.reciprocal_approx_fast(rb[:], c0[:])
                    ct = ctxp.tile([128, WQ], BF16, name=f"ct_{w}_{hp}", tag="ctx")
                    # h1 ctx rows 64:128 via partition-shift DMA
                    nc.sync.dma_start(ct[64:128, :], cr1[:])
                    nc.vector.tensor_mul(ct[0:64, :], cr0[:], rb[0:64, :])
                    nc.vector.tensor_mul(ct[64:128, :], ct[64:128, :], rb[64:128, :])
                    ctx_w.append(ct)
                for sc in range(4):
                    for n2 in range(2):
                        proj_q.append((("wo", w), gen_wo_group(w, sc, n2, ctx_w)))
            fill(1 << 30)
    nc.compile()
    return nc


def _causal_masks():
    """4 diagonal-offset 0/1 masks [128, 4*512] bf16 (keep iff j >= i + r*128)."""
    import ml_dtypes

    i = np.arange(128)[:, None]
    j = np.arange(WQ)[None, :]
    blocks = [(j >= (i + r * 128)).astype(np.float32) for r in range(4)]
    return np.concatenate(blocks, axis=1).astype(ml_dtypes.bfloat16)


def make_in_maps(x, W_q, b_q, W_k, b_k, W_v, b_v, W_o, b_o):
    mask = _causal_masks()
    scale = 1.0 / np.sqrt(DH)
    in_maps = []
    import ml_dtypes

    bf = ml_dtypes.bfloat16
    for core in range(N_CORES):
        b, g = core // 2, core % 2
        sl = slice(g * E, (g + 1) * E)
        in_maps.append(
            {
                "xt": np.ascontiguousarray(x[b].T).astype(bf),
                "wq": (np.ascontiguousarray(W_q[:, sl]) * np.float32(scale)).astype(bf),
                "wk": np.ascontiguousarray(W_k[:, sl]).astype(bf),
                "wv": np.ascontiguousarray(W_v[:, sl]).astype(bf),
                "wo": np.ascontiguousarray(W_o[sl, :]).astype(bf),
                "bq": np.ascontiguousarray(b_q[sl]) * np.float32(scale),
                "bk": np.ascontiguousarray(b_k[sl]),
                "mk": mask,
            }
        )
    return in_maps


def assemble(results, W_o, b_v, b_o):
    bo_eff = (b_o + b_v @ W_o).astype(np.float32)
    out = np.empty((B, S, D), dtype=np.float32)
    for b in range(B):
        out[b] = results[2 * b]["out"] + results[2 * b + 1]["out"] + bo_eff
    return out


def kernel(x, W_q, b_q, W_k, b_k, W_v, b_v, W_o, b_o, _trace=False):
    x = np.asarray(x, dtype=np.float32)
    W_q = np.asarray(W_q, dtype=np.float32)
    b_q = np.asarray(b_q, dtype=np.float32)
    W_k = np.asarray(W_k, dtype=np.float32)
    b_k = np.asarray(b_k, dtype=np.float32)
    W_v = np.asarray(W_v, dtype=np.float32)
    b_v = np.asarray(b_v, dtype=np.float32)
    W_o = np.asarray(W_o, dtype=np.float32)
    b_o = np.asarray(b_o, dtype=np.float32)

    if "nc" not in _cache:
        _cache["nc"] = build_program()
    nc = _cache["nc"]
    in_maps = make_in_maps(x, W_q, b_q, W_k, b_k, W_v, b_v, W_o, b_o)
    res = bass_utils.run_bass_kernel_spmd(
        nc, in_maps, core_ids=list(range(N_CORES)), trace=_trace
    )
    out = assemble(res.results, W_o, b_v, b_o)
    if _trace:
        return out, res
    return out
